# revision 3
# baseline (speedup 1.0000x reference)
"""Cross-attention kernel for Trainium2, 8 NeuronCores.

Reference computation (B=4, S=2048, C=1024, E=1024, D=768, H=16, hd=64):
    q = x @ q_w + q_b                 # [B,S,E]
    k = context @ k_w + k_b           # [B,C,E]
    v = context @ v_w + v_b           # [B,C,E]
    attn = softmax(q·k^T / sqrt(hd))  # per head
    out = (attn @ v) @ o_w + o_b      # [B,S,E]

Sharding: 8 cores = 4 batches x 2 head-groups (8 heads = 512 embed cols each).
Each core computes the full attention for its (batch, head-group) and a
partial out-projection; the host sums the two head-group partials per batch
(the "all-reduce") and adds o_b.

Device layout trick: everything is computed in a transposed orientation so no
on-device transposes are needed.  The host passes x^T and context^T; the
projections produce Q^T/K^T with the head dim on partitions and V in natural
layout.  Scores are computed transposed (S^T = K @ Q^T, contraction over
hd=64, two heads packed into the 128-row PE array via row groups), the
softmax denominator comes free from the attention@V matmul by appending a
ones column to V (stationary operand is [V_h | 1], M=65), and the final
normalization is a per-column multiply using a gpsimd partition-broadcast of
the reciprocal sums.  All matmuls run as float32r (fp22 multiply, fp32
accumulate) which is full-rate on the PE for 512-wide moving operands.
"""

import sys

sys.path.insert(0, "/opt/trn_rl_repo")

import numpy as np

B, S, E, C, D = 4, 2048, 1024, 1024, 768
H, HD = 16, 64
EL = E // 2          # embed columns per head-group (8 heads)
N_CORES = 8
NS = S // 512        # s-tiles of 512
KE = E // 128        # contraction chunks for q-proj
KD = D // 128        # contraction chunks for k/v-proj
NC2 = C // 512       # c-tiles of 512
CC = C // 128        # c chunks of 128
HP = EL // 128       # head pairs per core (4)

_built = None
_last_results = None


def _build():
    import concourse.bacc as bacc
    import concourse.mybir as mybir
    from concourse.tile import TileContext

    F32 = mybir.dt.float32
    F32R = mybir.dt.float32r
    Exp = mybir.ActivationFunctionType.Exp
    Ident = mybir.ActivationFunctionType.Identity

    nc = bacc.Bacc(None, target_bir_lowering=False)

    xT = nc.declare_dram_parameter("xT", [E, S], F32, isOutput=False)
    ctxT = nc.declare_dram_parameter("ctxT", [D, C], F32, isOutput=False)
    qw = nc.declare_dram_parameter("qw", [E, EL], F32, isOutput=False)
    kw = nc.declare_dram_parameter("kw", [D, EL], F32, isOutput=False)
    vw = nc.declare_dram_parameter("vw", [D, EL], F32, isOutput=False)
    ow = nc.declare_dram_parameter("ow", [EL, E], F32, isOutput=False)
    qb = nc.declare_dram_parameter("qb", [EL, 1], F32, isOutput=False)
    kb = nc.declare_dram_parameter("kb", [EL, 1], F32, isOutput=False)
    vb = nc.declare_dram_parameter("vb", [1, EL], F32, isOutput=False)
    ones8 = nc.declare_dram_parameter("ones8", [128, 8], F32, isOutput=False)
    out = nc.declare_dram_parameter("out", [S, E], F32, isOutput=True)

    def r(ap):
        return ap.bitcast(F32R)

    with TileContext(nc) as tc:
        with (
            tc.tile_pool(name="wpool", bufs=1) as wpool,
            tc.tile_pool(name="dpool", bufs=1) as dpool,
            tc.tile_pool(name="xpool", bufs=10) as xpool,
            tc.tile_pool(name="qtpool", bufs=8) as qtpool,
            tc.tile_pool(name="ptpool", bufs=6) as ptpool,
            tc.tile_pool(name="otpool", bufs=8) as otpool,
            tc.tile_pool(name="spool", bufs=3) as spool,
            tc.tile_pool(name="opool", bufs=4) as opool,
            tc.tile_pool(name="mmps", bufs=5, space="PSUM") as mmps,
            tc.tile_pool(name="ovps", bufs=2, space="PSUM") as ovps,
        ):
            # ---- weight / bias / context loads --------------------------------
            qw_sb = []
            for k in range(KE):
                t = wpool.tile([128, EL], F32R, name=f"qw{k}")
                nc.sync.dma_start(out=t[:], in_=r(qw[k * 128:(k + 1) * 128, :]))
                qw_sb.append(t)
            kw_sb, vw_sb = [], []
            for k in range(KD):
                t = wpool.tile([128, EL], F32R, name=f"kw{k}")
                nc.sync.dma_start(out=t[:], in_=r(kw[k * 128:(k + 1) * 128, :]))
                kw_sb.append(t)
                t = wpool.tile([128, EL], F32R, name=f"vw{k}")
                nc.sync.dma_start(out=t[:], in_=r(vw[k * 128:(k + 1) * 128, :]))
                vw_sb.append(t)
            ow_sb = []
            for k in range(HP):
                t = wpool.tile([128, E], F32R, name=f"ow{k}")
                nc.sync.dma_start(out=t[:], in_=r(ow[k * 128:(k + 1) * 128, :]))
                ow_sb.append(t)
            qb_sb, kb_sb = [], []
            for m in range(HP):
                t = wpool.tile([128, 1], F32, name=f"qb{m}")
                nc.sync.dma_start(out=t[:], in_=qb[m * 128:(m + 1) * 128, :])
                qb_sb.append(t)
                t = wpool.tile([128, 1], F32, name=f"kb{m}")
                nc.sync.dma_start(out=t[:], in_=kb[m * 128:(m + 1) * 128, :])
                kb_sb.append(t)
            vb_sb = wpool.tile([1, EL], F32, name="vb_sb")
            nc.sync.dma_start(out=vb_sb[:], in_=vb[:])
            vb_bc = wpool.tile([128, EL], F32, name="vb_bc")
            nc.gpsimd.partition_broadcast(vb_bc[:], vb_sb[:])

            ctx_sb = []
            for d in range(KD):
                t = dpool.tile([128, C], F32R, name=f"ctx{d}")
                nc.sync.dma_start(out=t[:], in_=r(ctxT[d * 128:(d + 1) * 128, :]))
                ctx_sb.append(t)

            # ---- K^T projection: [EL rows, C cols], head pairs on partitions --
            kt_sb = []
            for m in range(HP):
                t = dpool.tile([128, C], F32R, name=f"kt{m}")
                kt_sb.append(t)
                for t2 in range(NC2):
                    ps = mmps.tile([128, 512], F32, name="mm_ps", tag="mm")
                    for d in range(KD):
                        nc.tensor.matmul(
                            ps[:],
                            kw_sb[d][:, m * 128:(m + 1) * 128],
                            ctx_sb[d][:, t2 * 512:(t2 + 1) * 512],
                            start=(d == 0), stop=(d == KD - 1),
                        )
                    nc.scalar.activation(
                        t[:, t2 * 512:(t2 + 1) * 512], ps[:], Ident,
                        bias=kb_sb[m][:, 0:1],
                    )

            # ---- V projection: natural [C rows, EL cols], interleaved with a
            # ones column per head for the softmax denominator ------------------
            v_sb = []
            for mc in range(CC):
                t = dpool.tile([128, 8 * 65], F32R, name=f"v{mc}")
                v_sb.append(t)
                ps = mmps.tile([128, 512], F32, name="mm_ps", tag="mm")
                for d in range(KD):
                    nc.tensor.matmul(
                        ps[:],
                        ctx_sb[d][:, mc * 128:(mc + 1) * 128],
                        vw_sb[d][:],
                        start=(d == 0), stop=(d == KD - 1),
                    )
                vv = t.rearrange("p (h u) -> p h u", u=65)
                nc.vector.tensor_add(
                    vv[:, :, 0:64],
                    ps.rearrange("p (h u) -> p h u", u=64),
                    vb_bc.rearrange("p (h u) -> p h u", u=64),
                )
                nc.sync.dma_start(
                    out=vv[:, :, 64:65],
                    in_=r(ones8[:, :]).rearrange("p (h u) -> p h u", u=1),
                )

            # ---- main loop over s-tiles of 512 --------------------------------
            for n in range(NS):
                # x^T tiles for this s-tile
                xts = []
                for k in range(KE):
                    t = xpool.tile([128, 512], F32R, name="xt", tag="xt")
                    nc.sync.dma_start(
                        out=t[:],
                        in_=r(xT[k * 128:(k + 1) * 128, n * 512:(n + 1) * 512]),
                    )
                    xts.append(t)
                # Q^T projection (scale folded into qw/qb on host)
                qts = []
                for m in range(HP):
                    ps = mmps.tile([128, 512], F32, name="mm_ps", tag="mm")
                    for k in range(KE):
                        nc.tensor.matmul(
                            ps[:],
                            qw_sb[k][:, m * 128:(m + 1) * 128],
                            xts[k][:],
                            start=(k == 0), stop=(k == KE - 1),
                        )
                    qt_t = qtpool.tile([128, 512], F32R, name="qt", tag="qt")
                    nc.scalar.activation(qt_t[:], ps[:], Ident, bias=qb_sb[m][:, 0:1])
                    qts.append(qt_t)

                # attention, one head pair at a time
                ots = []
                for hp in range(HP):
                    ovs = [
                        ovps.tile([65, 512], F32, name="ov", tag="ov")
                        for _ in range(2)
                    ]
                    for c in range(CC):
                        pts = []
                        for h2 in range(2):
                            sc = mmps.tile([128, 512], F32, name="mm_ps", tag="mm")
                            # scores^T block: K_h @ Q_h^T, contraction hd=64.
                            # h2=0 uses PE rows 0-63, h2=1 rows 64-127 -> the two
                            # matmuls run concurrently in different row groups.
                            nc.tensor.matmul(
                                sc[:],
                                kt_sb[hp][h2 * 64:(h2 + 1) * 64, c * 128:(c + 1) * 128],
                                qts[hp][h2 * 64:(h2 + 1) * 64, :],
                                start=True, stop=True,
                            )
                            p = ptpool.tile([128, 512], F32R, name="pt", tag="pt")
                            nc.scalar.activation(p[:], sc[:], Exp)
                            pts.append(p)
                        for h2 in range(2):
                            h = hp * 2 + h2
                            nc.tensor.matmul(
                                ovs[h2][:],
                                v_sb[c][:, h * 65:(h + 1) * 65],
                                pts[h2][:],
                                start=(c == 0), stop=(c == CC - 1),
                            )
                    ot_t = otpool.tile([128, 512], F32R, name="ot", tag="ot")
                    for h2 in range(2):
                        rs = spool.tile([1, 512], F32, name="rs", tag="rs")
                        nc.vector.reciprocal(rs[:], ovs[h2][64:65, :])
                        bc = spool.tile([64, 512], F32, name="bc", tag="bc")
                        nc.gpsimd.partition_broadcast(bc[:], rs[:])
                        nc.vector.tensor_mul(
                            ot_t[h2 * 64:(h2 + 1) * 64, :], ovs[h2][0:64, :], bc[:]
                        )
                    ots.append(ot_t)

                # out-projection for this s-tile (partial over this head group)
                for ss in range(4):
                    for ne in range(2):
                        ps = mmps.tile([128, 512], F32, name="mm_ps", tag="mm")
                        for hp in range(HP):
                            nc.tensor.matmul(
                                ps[:],
                                ots[hp][:, ss * 128:(ss + 1) * 128],
                                ow_sb[hp][:, ne * 512:(ne + 1) * 512],
                                start=(hp == 0), stop=(hp == HP - 1),
                            )
                        o_sb = opool.tile([128, 512], F32, name="o_sb", tag="o")
                        nc.vector.tensor_copy(o_sb[:], ps[:])
                        nc.sync.dma_start(
                            out=out[n * 512 + ss * 128:n * 512 + (ss + 1) * 128,
                                    ne * 512:(ne + 1) * 512],
                            in_=o_sb[:],
                        )

    nc.finalize()
    return nc


def kernel(x, context, q_w, q_b, k_w, k_b, v_w, v_b, o_w, o_b):
    global _built, _last_results
    from concourse.bass_utils import run_bass_kernel_spmd

    if _built is None:
        _built = _build()
    nc = _built

    scale = np.float32(1.0 / np.sqrt(HD))
    x = np.asarray(x, np.float32)
    context = np.asarray(context, np.float32)
    xTs = [np.ascontiguousarray(x[b].T) for b in range(B)]
    ctxTs = [np.ascontiguousarray(context[b].T) for b in range(B)]

    in_maps = []
    for core in range(N_CORES):
        b, hg = core // 2, core % 2
        el = slice(hg * EL, (hg + 1) * EL)
        in_maps.append({
            "xT": xTs[b],
            "ctxT": ctxTs[b],
            "qw": np.ascontiguousarray(np.asarray(q_w, np.float32)[:, el] * scale),
            "kw": np.ascontiguousarray(np.asarray(k_w, np.float32)[:, el]),
            "vw": np.ascontiguousarray(np.asarray(v_w, np.float32)[:, el]),
            "ow": np.ascontiguousarray(np.asarray(o_w, np.float32)[el, :]),
            "qb": np.ascontiguousarray(
                (np.asarray(q_b, np.float32)[el] * scale)[:, None]),
            "kb": np.ascontiguousarray(np.asarray(k_b, np.float32)[el][:, None]),
            "vb": np.ascontiguousarray(np.asarray(v_b, np.float32)[el][None, :]),
            "ones8": np.ones((128, 8), np.float32),
        })

    res = run_bass_kernel_spmd(nc, in_maps, list(range(N_CORES)))
    _last_results = res

    ob = np.asarray(o_b, np.float32)
    full = np.empty((B, S, E), np.float32)
    for b in range(B):
        full[b] = res.results[2 * b]["out"] + res.results[2 * b + 1]["out"] + ob
    return full
